# revision 7
# baseline (speedup 1.0000x reference)
"""Trainium2 Bass kernel for nn_ModelNew_78847009620052 (dense_mlp).

Computes, for x [4096, 8192] and weight [8192, 8192]:
    out[b, 0] = 0.75 * sum_i x[b, i] * (sum_j weight[j, i])
(which equals 1.5 * sum(x @ W.T / 2, axis=1, keepdims=True)).

Sharding: column-shard the contraction dim IN=8192 into 8 chunks of 1024.
Core d receives x[:, d*1024:(d+1)*1024] and weight[:, d*1024:(d+1)*1024],
produces a partial [128, 32] result; host sums the 8 partials (after a
[128,32] -> [4096,1] reindex).

The kernel is a pure HBM-bandwidth problem (every input byte is read
once, ~408 GB/s/core sustained).  The single biggest lever is the byte
count: the host casts both inputs to float16 before upload, halving
per-core traffic from 48MB to 24MB (~123us -> ~62us of streaming).
Accumulation stays in fp32 (PSUM matmul accumulate + fp32 accum_out),
so the end-to-end error is ~5e-4 relative -- 40x inside the 2e-2 gate
(fp16 keeps 11 mantissa bits; x, w ~ N(0,1), all intermediates are
orders of magnitude below fp16's 65504 max).

Per-core device algorithm:
  Phase 1 (weight, 16MB fp16): stream [P, t, 1024] tiles (t<=8, 2MB
    DMAs).  Per pair of row-tiles: ONE VectorE fp16 in-tile add folds
    the pair, then ONE TensorE fp16 matmul pair (ones*0.75 stationary)
    accumulates the folded tile's column sums into PSUM fp32, broadcast
    to all 128 partitions.  No cross-unit add trees: every unit's
    serial chain is land -> add -> matmul pair, so the reduction never
    lags the DMA stream.  The last two units are single row-tiles
    consumed by a direct matmul pair, keeping the stop chain minimal.
    After the stop matmul one VectorE copy casts the PSUM column sums
    to an SBUF fp16 operand tile.
  Phase 2 (x, 8MB fp16): one fused mul+reduce per row-tile on VectorE:
        scalar_tensor_tensor: scratch = x_tile * wsum16,
                              accum_out(s_col, fp32) = row sums
    ~0.6us per [128,1024] fp16 row-tile; 32 tiles just fit inside the
    ~21us x DMA window.  ScalarE does no compute; it only issues the
    two output stores (on the Activation HWDGE queue, so the in-order
    SP sequencer never blocks later x DMA issues on a store's wait).

All DMAs are HWDGE, weight queued strictly before x, so the SDMA
engines drain the weight stream at full HBM rate and the x stream
follows seamlessly.  Small tail units keep the post-stream serial
chain (last STT -> store -> teardown) to a few microseconds.
"""

import numpy as np

B, IN, HID = 4096, 8192, 8192
N_CORES = 8
CHUNK = IN // N_CORES          # 1024 columns per core
SCALE = 1.5 / 2.0              # 0.75, folded into the ones stationary
P = 128                        # partitions
W_TILES = HID // P             # 64 weight row-tiles per core
X_TILES = B // P               # 32 x row-tiles per core

# Row-tiles per DMA ([P, t, 1024] fp16 = t * 256KB per DMA).
W_UNITS = [4, 4] + [8] * 6 + [4, 2, 1, 1]
X_UNITS = [8, 8, 8, 4, 2, 1, 1]
assert sum(W_UNITS) == W_TILES and sum(X_UNITS) == X_TILES

S_SPLIT = 24                   # first store covers s columns [0, 24)

_compiled_nc = None


def _build_nc():
    import concourse.bass as bass
    import concourse.tile as tile
    from concourse import bacc, mybir

    f32 = mybir.dt.float32
    f16 = mybir.dt.float16
    nc = bacc.Bacc(
        "TRN2",
        target_bir_lowering=False,
        debug=False,
        num_devices=N_CORES,
    )

    x_d = nc.dram_tensor("x", [B, CHUNK], f16, kind="ExternalInput")
    w_d = nc.dram_tensor("w", [HID, CHUNK], f16, kind="ExternalInput")
    out_d = nc.dram_tensor("out", [P, X_TILES], f32, kind="ExternalOutput")

    with tile.TileContext(nc) as tc:
        with (
            tc.tile_pool(name="wpool", bufs=3) as wpool,
            tc.tile_pool(name="xpool", bufs=3) as xpool,
            tc.tile_pool(name="const", bufs=1) as const,
            tc.tile_pool(name="psum", bufs=1, space="PSUM") as psum_pool,
        ):
            # Warmup: tiny DMA issued first to probe/shrink HWDGE cold start.
            warm = const.tile([P, 16], f16)
            nc.sync.dma_start(warm[:], w_d[0:P, 0:16])

            ones = const.tile([P, P], f16)
            nc.vector.memset(ones[:], SCALE)

            # Column sums accumulate in PSUM fp32, broadcast to all 128
            # partitions by the ones matmul.
            psum_bc = psum_pool.tile([P, CHUNK], f32, tag="psum_bc")

            first_mm = [True]

            def colsum_pair(src_ap, stop):
                for h in range(2):
                    nc.tensor.matmul(
                        psum_bc[:, h * 512 : (h + 1) * 512],
                        ones[:],
                        src_ap[:, h * 512 : (h + 1) * 512],
                        start=first_mm[0],
                        stop=stop,
                    )
                first_mm[0] = False

            # --- Phase 1: weight stream. Per pair of row-tiles: one DVE
            # fold + one PE pair; single-row-tile units go straight to PE.
            row = 0
            for ui, t in enumerate(W_UNITS):
                last_unit = ui == len(W_UNITS) - 1
                wt = wpool.tile(
                    [P, t, CHUNK], f16, tag=f"w{t}",
                    bufs=(3 if t == 8 else 2),
                )
                src = w_d[row * P : (row + t) * P, :]
                nc.sync.dma_start(
                    wt[:], src.rearrange("(t p) c -> p t c", p=P)
                )
                if t == 1:
                    colsum_pair(wt[:, 0, :], stop=last_unit)
                else:
                    for k in range(t // 2):
                        nc.vector.tensor_add(
                            wt[:, 2 * k, :], wt[:, 2 * k, :], wt[:, 2 * k + 1, :]
                        )
                        colsum_pair(wt[:, 2 * k, :], stop=False)
                row += t
            assert row == W_TILES

            # Cast the broadcast column sums to an SBUF fp16 operand so
            # phase-2 STTs run at the 16-bit DVE rate.
            wsum16 = const.tile([P, CHUNK], f16)
            nc.vector.tensor_copy(wsum16[:], psum_bc[:])

            # --- Phase 2: x stream. Per unit: ONE wide in-place DVE mul
            # (stride-0 broadcast operand), then the unit's row sums split
            # between ScalarE (leading tiles, one activation-accum each)
            # and VectorE (trailing tiles, one fused multi-column
            # reduce_sum), both writing f32 s columns directly.  Work is
            # balanced so DVE (~21us) and ACT (~20us) both track the
            # ~21us x DMA window.
            s_a = const.tile([P, 16], f32)           # row-tiles 0-15
            s_b = const.tile([P, X_TILES - 16], f32)  # row-tiles 16-31
            act_out = const.tile([P, CHUNK], f16)

            def s_tile(idx):
                return (s_a, idx) if idx < 16 else (s_b, idx - 16)

            # Per-unit count of leading tiles reduced on ScalarE.
            ACT_TILES = {8: 4, 4: 2, 2: 1, 1: 0}

            row = 0
            for t in X_UNITS:
                xt = xpool.tile(
                    [P, t, CHUNK], f16, tag=f"x{t}",
                    bufs=(3 if t == 8 else 2),
                )
                src = x_d[row * P : (row + t) * P, :]
                nc.sync.dma_start(
                    xt[:], src.rearrange("(t p) c -> p t c", p=P)
                )
                nc.vector.tensor_mul(
                    xt[:],
                    xt[:],
                    wsum16[:].unsqueeze(1).broadcast_to((P, t, CHUNK)),
                )
                n_act = ACT_TILES[t]
                for k in range(n_act):
                    st, c = s_tile(row + k)
                    nc.scalar.activation(
                        act_out[:],
                        xt[:, k, :],
                        mybir.ActivationFunctionType.Copy,
                        bias=0.0,
                        scale=1.0,
                        accum_out=st[:, c : c + 1],
                    )
                st, c = s_tile(row + n_act)
                assert c + (t - n_act) <= 16
                nc.vector.reduce_sum(
                    st[:, c : c + (t - n_act)],
                    xt[:, n_act:t, :],
                    axis=mybir.AxisListType.X,
                )
                row += t
                if row == 16:
                    # Row-tiles 0-15 complete: store them while the tail
                    # still streams (Activation HWDGE queue keeps SP free).
                    nc.scalar.dma_start(out_d[:, 0:16], s_a[:])
            assert row == X_TILES

            nc.scalar.dma_start(out_d[:, 16:X_TILES], s_b[:])

    nc.compile()
    return nc


def _get_nc():
    global _compiled_nc
    if _compiled_nc is None:
        _compiled_nc = _build_nc()
    return _compiled_nc


def kernel(x: np.ndarray, weight: np.ndarray) -> np.ndarray:
    from concourse.bass_utils import run_bass_kernel_spmd

    x = np.asarray(x)
    weight = np.asarray(weight)
    assert x.shape == (B, IN) and weight.shape == (HID, IN)
    x16 = x.astype(np.float16)
    w16 = weight.astype(np.float16)

    nc = _get_nc()
    in_maps = [
        {
            "x": np.ascontiguousarray(x16[:, d * CHUNK : (d + 1) * CHUNK]),
            "w": np.ascontiguousarray(w16[:, d * CHUNK : (d + 1) * CHUNK]),
        }
        for d in range(N_CORES)
    ]
    res = run_bass_kernel_spmd(nc, in_maps, core_ids=list(range(N_CORES)))
    acc = np.zeros((B, 1), dtype=np.float64)
    for d in range(N_CORES):
        acc += res.results[d]["out"].T.reshape(B, 1).astype(np.float64)
    return acc.astype(np.float32)


# revision 12
# speedup vs baseline: 1.0291x; 1.0291x over previous
"""Trainium2 Bass kernel for nn_ModelNew_78847009620052 (dense_mlp).

Computes, for x [4096, 8192] and weight [8192, 8192]:
    out[b, 0] = 0.75 * sum_i x[b, i] * (sum_j weight[j, i])
(which equals 1.5 * sum(x @ W.T / 2, axis=1, keepdims=True)).

Sharding: column-shard the contraction dim IN=8192 into 8 chunks of 1024.
Core d receives x[:, d*1024:(d+1)*1024] and weight[:, d*1024:(d+1)*1024],
produces a partial [128, 32] result; host sums the 8 partials (after a
[128,32] -> [4096,1] reindex).

The kernel is a pure HBM-bandwidth problem (every input byte is read
once, ~408 GB/s/core sustained).  The single biggest lever is the byte
count: the host casts both inputs to float16 before upload, halving
per-core traffic from 48MB to 24MB (~123us -> ~62us of streaming).
Accumulation stays in fp32 (PSUM matmul accumulate + fp32 accum_out),
so the end-to-end error is ~5e-4 relative -- 40x inside the 2e-2 gate
(fp16 keeps 11 mantissa bits; x, w ~ N(0,1), all intermediates are
orders of magnitude below fp16's 65504 max).

Per-core device algorithm:
  Phase 1 (weight, 16MB fp16): stream [P, t, 1024] tiles (t<=8, 2MB
    DMAs).  Per pair of row-tiles: ONE VectorE fp16 in-tile add folds
    the pair, then ONE TensorE fp16 matmul pair (ones*0.75 stationary)
    accumulates the folded tile's column sums into PSUM fp32, broadcast
    to all 128 partitions.  No cross-unit add trees: every unit's
    serial chain is land -> add -> matmul pair, so the reduction never
    lags the DMA stream.  The last two units are single row-tiles
    consumed by a direct matmul pair, keeping the stop chain minimal.
    After the stop matmul one VectorE copy casts the PSUM column sums
    to an SBUF fp16 operand tile.
  Phase 2 (x, 8MB fp16): every ACCUMULATING op on this HW runs at
    ~1 elem/cycle/lane regardless of dtype (DVE scalar_tensor_tensor
    ~1.22us, DVE reduce_sum ~1.05ns/elem, ScalarE activation-accum
    ~1.33us per [128,1024] tile), while plain fp16 elementwise ops run
    at 2x.  GpSimd is useless here: its tensor ops are 2.5x slower AND
    fight VectorE for the shared SBUF port (measured: concurrent gp
    muls degrade DVE STTs 1.22us -> 3.0us).  So the row sums are split
    across the two usable engines:
      - leading tiles per unit: one fused in-place DVE fp16 mul
        (x *= colsums, ~0.53ns/elem) + one ScalarE activation-accum
        per tile;
      - trailing tiles: one DVE scalar_tensor_tensor (fused
        mul+reduce) per tile, straight from the raw x tile;
    with the tile split chosen so both engines finish together
    (ScalarE starts ~8us later, gated by the first mul).

All DMAs are HWDGE, weight queued strictly before x, so the SDMA
engines drain the weight stream at full HBM rate and the x stream
follows seamlessly.  Output stores are issued from the Activation
HWDGE queue so the in-order SP sequencer never blocks later x DMA
issues on a store's semaphore wait.  Small tail units keep the
post-stream serial chain (last STT -> store -> teardown) short.
"""

import numpy as np

B, IN, HID = 4096, 8192, 8192
N_CORES = 8
CHUNK = IN // N_CORES          # 1024 columns per core
SCALE = 1.5 / 2.0              # 0.75, folded into the ones stationary
P = 128                        # partitions
W_TILES = HID // P             # 64 weight row-tiles per core
X_TILES = B // P               # 32 x row-tiles per core

# Row-tiles per DMA ([P, t, 1024] fp16 = t * 256KB per DMA).
W_UNITS = [4, 4] + [8] * 6 + [4, 2, 1, 1]
X_UNITS = [8, 8, 8, 4, 2, 1, 1]
assert sum(W_UNITS) == W_TILES and sum(X_UNITS) == X_TILES

S_SPLIT = 24                   # first store covers s columns [0, 24)

_compiled_nc = None


def _build_nc():
    import concourse.bass as bass
    import concourse.tile as tile
    from concourse import bacc, mybir

    f32 = mybir.dt.float32
    f16 = mybir.dt.float16
    nc = bacc.Bacc(
        "TRN2",
        target_bir_lowering=False,
        debug=False,
        num_devices=N_CORES,
    )

    x_d = nc.dram_tensor("x", [B, CHUNK], f16, kind="ExternalInput")
    w_d = nc.dram_tensor("w", [HID, CHUNK], f16, kind="ExternalInput")
    out_d = nc.dram_tensor("out", [P, X_TILES], f32, kind="ExternalOutput")

    with tile.TileContext(nc) as tc:
        with (
            tc.tile_pool(name="wpool", bufs=3) as wpool,
            tc.tile_pool(name="xpool", bufs=3) as xpool,
            tc.tile_pool(name="const", bufs=1) as const,
            tc.tile_pool(name="psum", bufs=1, space="PSUM") as psum_pool,
        ):
            # Warmup: tiny DMA issued first to probe/shrink HWDGE cold start.
            warm = const.tile([P, 16], f16)
            nc.sync.dma_start(warm[:], w_d[0:P, 0:16])

            ones = const.tile([P, P], f16)
            nc.vector.memset(ones[:], SCALE)

            # Column sums accumulate in PSUM fp32, broadcast to all 128
            # partitions by the ones matmul.
            psum_bc = psum_pool.tile([P, CHUNK], f32, tag="psum_bc")

            first_mm = [True]

            def colsum_pair(src_ap, stop):
                for h in range(2):
                    nc.tensor.matmul(
                        psum_bc[:, h * 512 : (h + 1) * 512],
                        ones[:],
                        src_ap[:, h * 512 : (h + 1) * 512],
                        start=first_mm[0],
                        stop=stop,
                    )
                first_mm[0] = False

            # --- Phase 1: weight stream. Per pair of row-tiles: one DVE
            # fold + one PE pair; single-row-tile units go straight to PE.
            row = 0
            for ui, t in enumerate(W_UNITS):
                last_unit = ui == len(W_UNITS) - 1
                wt = wpool.tile(
                    [P, t, CHUNK], f16, tag=f"w{t}",
                    bufs=(3 if t == 8 else 2),
                )
                src = w_d[row * P : (row + t) * P, :]
                nc.sync.dma_start(
                    wt[:], src.rearrange("(t p) c -> p t c", p=P)
                )
                if t == 1:
                    colsum_pair(wt[:, 0, :], stop=last_unit)
                else:
                    for k in range(t // 2):
                        nc.vector.tensor_add(
                            wt[:, 2 * k, :], wt[:, 2 * k, :], wt[:, 2 * k + 1, :]
                        )
                        colsum_pair(wt[:, 2 * k, :], stop=False)
                row += t
            assert row == W_TILES

            # Cast the broadcast column sums to an SBUF fp16 operand so
            # phase-2 STTs run at the 16-bit DVE rate.
            wsum16 = const.tile([P, CHUNK], f16)
            nc.vector.tensor_copy(wsum16[:], psum_bc[:])

            # --- Phase 2: x stream. Every accumulating op on this HW runs
            # at ~1 elem/cycle/lane regardless of dtype (measured: DVE STT
            # ~1.22us, DVE reduce ~1.05ns/elem, ACT accum ~1.33us per
            # [128,1024] tile), while plain fp16 elementwise ops run at 2x
            # (~0.6ns/elem). The optimal 2-engine split is therefore:
            #   per unit, the leading n_act tiles: ONE fused in-place DVE
            #     mul (x *= colsums, ~0.53ns/elem), then one ScalarE
            #     activation-accum per tile (1.33us);
            #   the trailing tiles: one DVE STT (fused mul+reduce, 1.22us)
            #     each, straight from the raw x tile.
            # With 20 ACT tiles / 12 STT tiles both engines carry ~27us.
            s_a = const.tile([P, 16], f32)            # row-tiles 0-15
            s_b = const.tile([P, X_TILES - 16], f32)  # row-tiles 16-31
            act_out = const.tile([P, CHUNK], f16)
            scratch = const.tile([P, CHUNK], f16)

            def s_col(idx):
                return (
                    s_a[:, idx : idx + 1]
                    if idx < 16
                    else s_b[:, idx - 16 : idx - 15]
                )

            # Per-unit count of leading tiles reduced on ScalarE
            # (remainder go through DVE STT). ACT starts ~8us after DVE
            # (it waits on the first mul), so it carries fewer tiles for
            # both engines to finish together.
            ACT_TILES = {8: 4, 4: 2, 2: 0, 1: 0}

            row = 0
            for t in X_UNITS:
                xt = xpool.tile(
                    [P, t, CHUNK], f16, tag=f"x{t}",
                    bufs=(3 if t == 8 else 2),
                )
                src = x_d[row * P : (row + t) * P, :]
                nc.sync.dma_start(
                    xt[:], src.rearrange("(t p) c -> p t c", p=P)
                )
                n_act = ACT_TILES[t]
                if n_act:
                    nc.vector.tensor_mul(
                        xt[:, 0:n_act, :],
                        xt[:, 0:n_act, :],
                        wsum16[:].unsqueeze(1).broadcast_to((P, n_act, CHUNK)),
                    )
                for k in range(n_act):
                    nc.scalar.activation(
                        act_out[:],
                        xt[:, k, :],
                        mybir.ActivationFunctionType.Copy,
                        bias=0.0,
                        scale=1.0,
                        accum_out=s_col(row + k),
                    )
                for k in range(n_act, t):
                    nc.vector.scalar_tensor_tensor(
                        scratch[:],
                        xt[:, k, :],
                        0.0,
                        wsum16[:],
                        op0=mybir.AluOpType.bypass,
                        op1=mybir.AluOpType.mult,
                        accum_out=s_col(row + k),
                    )
                row += t
                if row == 16:
                    # Row-tiles 0-15 complete: store them while the tail
                    # still streams (Activation HWDGE queue keeps SP free).
                    nc.scalar.dma_start(out_d[:, 0:16], s_a[:])
            assert row == X_TILES

            nc.scalar.dma_start(out_d[:, 16:X_TILES], s_b[:])

    nc.compile()
    return nc


def _get_nc():
    global _compiled_nc
    if _compiled_nc is None:
        _compiled_nc = _build_nc()
    return _compiled_nc


def kernel(x: np.ndarray, weight: np.ndarray) -> np.ndarray:
    from concourse.bass_utils import run_bass_kernel_spmd

    x = np.asarray(x)
    weight = np.asarray(weight)
    assert x.shape == (B, IN) and weight.shape == (HID, IN)
    x16 = x.astype(np.float16)
    w16 = weight.astype(np.float16)

    nc = _get_nc()
    in_maps = [
        {
            "x": np.ascontiguousarray(x16[:, d * CHUNK : (d + 1) * CHUNK]),
            "w": np.ascontiguousarray(w16[:, d * CHUNK : (d + 1) * CHUNK]),
        }
        for d in range(N_CORES)
    ]
    res = run_bass_kernel_spmd(nc, in_maps, core_ids=list(range(N_CORES)))
    acc = np.zeros((B, 1), dtype=np.float64)
    for d in range(N_CORES):
        acc += res.results[d]["out"].T.reshape(B, 1).astype(np.float64)
    return acc.astype(np.float32)


# revision 17
# speedup vs baseline: 1.1149x; 1.0833x over previous
"""Trainium2 Bass kernel for nn_ModelNew_78847009620052 (dense_mlp).

Computes, for x [4096, 8192] and weight [8192, 8192]:
    out[b, 0] = 0.75 * sum_i x[b, i] * (sum_j weight[j, i])
(which equals 1.5 * sum(x @ W.T / 2, axis=1, keepdims=True)).

Sharding: column-shard the contraction dim IN=8192 into 8 chunks of 1024.
Core d receives x[:, d*1024:(d+1)*1024] and weight[:, d*1024:(d+1)*1024],
produces a partial [128, 32] result; host sums the 8 partials (after a
[128,32] -> [4096,1] reindex).

The kernel is a pure HBM-bandwidth problem (every input byte is read
once, ~408 GB/s/core sustained).  The single biggest lever is the byte
count: the host casts both inputs to float16 before upload, halving
per-core traffic from 48MB to 24MB (~123us -> ~62us of streaming).
Accumulation stays in fp32 (PSUM matmul accumulate + fp32 accum_out),
so the end-to-end error is ~5e-4 relative -- 40x inside the 2e-2 gate
(fp16 keeps 11 mantissa bits; x, w ~ N(0,1), all intermediates are
orders of magnitude below fp16's 65504 max).

Per-core device algorithm:
  Phase 1 (weight, 16MB fp16): stream [P, t, 1024] tiles (t<=8, 2MB
    DMAs).  Per pair of row-tiles: ONE VectorE fp16 in-tile add folds
    the pair, then ONE TensorE fp16 matmul pair (ones*0.75 stationary)
    accumulates the folded tile's column sums into PSUM fp32, broadcast
    to all 128 partitions.  No cross-unit add trees: every unit's
    serial chain is land -> add -> matmul pair, so the reduction never
    lags the DMA stream.  The last two units are single row-tiles
    consumed by a direct matmul pair, keeping the stop chain minimal.
    After the stop matmul one VectorE copy casts the PSUM column sums
    to an SBUF fp16 operand tile.
  Phase 2 (x, 8MB fp16): every ACCUMULATING op on this HW runs at
    ~1 elem/cycle/lane regardless of dtype (DVE scalar_tensor_tensor
    ~1.22us, DVE reduce_sum ~1.05ns/elem, ScalarE activation-accum
    ~1.33us per [128,1024] tile), while plain fp16 elementwise ops run
    at 2x.  GpSimd is useless here: its tensor ops are 2.5x slower AND
    fight VectorE for the shared SBUF port (measured: concurrent gp
    muls degrade DVE STTs 1.22us -> 3.0us).  So the row sums are split
    across the two usable engines:
      - leading tiles per unit: one fused in-place DVE fp16 mul
        (x *= colsums, ~0.53ns/elem) + one ScalarE activation-accum
        per tile;
      - trailing tiles: one DVE scalar_tensor_tensor (fused
        mul+reduce) per tile, straight from the raw x tile;
    with the tile split chosen so both engines finish together
    (ScalarE starts ~8us later, gated by the first mul).

All DMAs are HWDGE, weight queued strictly before x, so the SDMA
engines drain the weight stream at full HBM rate and the x stream
follows seamlessly.  Output stores are issued from the Activation
HWDGE queue so the in-order SP sequencer never blocks later x DMA
issues on a store's semaphore wait.  Small tail units keep the
post-stream serial chain (last STT -> store -> teardown) short.
"""

import numpy as np

B, IN, HID = 4096, 8192, 8192
N_CORES = 8
CHUNK = IN // N_CORES          # 1024 columns per core
SCALE = 1.5 / 2.0              # 0.75, folded into the ones stationary
P = 128                        # partitions
W_TILES = HID // P             # 64 weight row-tiles per core
X_TILES = B // P               # 32 x row-tiles per core

# Row-tiles per DMA ([P, t, 1024] fp16 = t * 256KB per DMA).  Tiles are
# loaded "(p t) c -> p t c": partition p holds t CONTIGUOUS rows
# (p*t+k), so every DMA descriptor line is a single t*2KB contiguous
# run (vs 2KB strided runs with the (t p) layout) — measurably better
# HBM efficiency.  The host gather undoes the batch permutation.
# The x stream leads with a small unit so the first VectorE STTs start
# as soon as the column sums are ready instead of waiting ~5us for a
# full 2MB tile.
W_UNITS = [4, 4] + [8] * 6 + [4, 2, 1, 1]
X_UNITS = [2, 8, 8, 8, 4, 1, 1]
assert sum(W_UNITS) == W_TILES and sum(X_UNITS) == X_TILES

S_SPLIT = 24                   # first store covers s columns [0, 24)

_compiled_nc = None


def _build_nc():
    import concourse.bass as bass
    import concourse.tile as tile
    from concourse import bacc, mybir

    f32 = mybir.dt.float32
    f16 = mybir.dt.float16
    nc = bacc.Bacc(
        "TRN2",
        target_bir_lowering=False,
        debug=False,
        num_devices=N_CORES,
    )

    x_d = nc.dram_tensor("x", [B, CHUNK], f16, kind="ExternalInput")
    w_d = nc.dram_tensor("w", [HID, CHUNK], f16, kind="ExternalInput")
    out_d = nc.dram_tensor("out", [P, X_TILES], f32, kind="ExternalOutput")

    with tile.TileContext(nc) as tc:
        with (
            tc.tile_pool(name="wpool", bufs=3) as wpool,
            tc.tile_pool(name="xpool", bufs=3) as xpool,
            tc.tile_pool(name="const", bufs=1) as const,
            tc.tile_pool(name="psum", bufs=1, space="PSUM") as psum_pool,
        ):
            # Warmup: tiny DMA issued first to probe/shrink HWDGE cold start.
            warm = const.tile([P, 16], f16)
            nc.sync.dma_start(warm[:], w_d[0:P, 0:16])

            ones = const.tile([P, P], f16)
            nc.vector.memset(ones[:], SCALE)

            # Column sums accumulate in PSUM fp32, broadcast to all 128
            # partitions by the ones matmul.
            psum_bc = psum_pool.tile([P, CHUNK], f32, tag="psum_bc")

            first_mm = [True]

            def colsum_pair(src_ap, stop):
                for h in range(2):
                    nc.tensor.matmul(
                        psum_bc[:, h * 512 : (h + 1) * 512],
                        ones[:],
                        src_ap[:, h * 512 : (h + 1) * 512],
                        start=first_mm[0],
                        stop=stop,
                    )
                first_mm[0] = False

            # --- Phase 1: weight stream. Per pair of row-tiles: one DVE
            # fold + one PE pair; single-row-tile units go straight to PE.
            row = 0
            for ui, t in enumerate(W_UNITS):
                last_unit = ui == len(W_UNITS) - 1
                wt = wpool.tile(
                    [P, t, CHUNK], f16, tag=f"w{t}",
                    bufs=(3 if t == 8 else 2),
                )
                src = w_d[row * P : (row + t) * P, :]
                nc.sync.dma_start(
                    wt[:], src.rearrange("(p t) c -> p t c", p=P)
                )
                if t == 1:
                    colsum_pair(wt[:, 0, :], stop=last_unit)
                else:
                    for k in range(t // 2):
                        nc.vector.tensor_add(
                            wt[:, 2 * k, :], wt[:, 2 * k, :], wt[:, 2 * k + 1, :]
                        )
                        colsum_pair(wt[:, 2 * k, :], stop=False)
                row += t
            assert row == W_TILES

            # Cast the broadcast column sums to an SBUF fp16 operand so
            # phase-2 STTs run at the 16-bit DVE rate.
            wsum16 = const.tile([P, CHUNK], f16)
            nc.vector.tensor_copy(wsum16[:], psum_bc[:])

            # --- Phase 2: x stream. Every accumulating op on this HW runs
            # at ~1 elem/cycle/lane regardless of dtype (measured: DVE STT
            # ~1.22us, DVE reduce ~1.05ns/elem, ACT accum ~1.33us per
            # [128,1024] tile), while plain fp16 elementwise ops run at 2x
            # (~0.6ns/elem). The optimal 2-engine split is therefore:
            #   per unit, the leading n_act tiles: ONE fused in-place DVE
            #     mul (x *= colsums, ~0.53ns/elem), then one ScalarE
            #     activation-accum per tile (1.33us);
            #   the trailing tiles: one DVE STT (fused mul+reduce, 1.22us)
            #     each, straight from the raw x tile.
            # 14 ACT tiles / 18 STT tiles makes both engines finish
            # together (ACT starts later; see ACT_TILES below).
            s_a = const.tile([P, 16], f32)            # row-tiles 0-15
            s_b = const.tile([P, X_TILES - 16], f32)  # row-tiles 16-31
            act_out = const.tile([P, CHUNK], f16)
            scratch = const.tile([P, CHUNK], f16)

            def s_col(idx):
                return (
                    s_a[:, idx : idx + 1]
                    if idx < 16
                    else s_b[:, idx - 16 : idx - 15]
                )

            # Per-unit count of leading tiles reduced on ScalarE
            # (remainder go through DVE STT). ACT starts ~8us after DVE
            # (it waits on the first mul), so it carries fewer tiles for
            # both engines to finish together.
            ACT_TILES = {8: 4, 4: 2, 2: 2, 1: 0}

            row = 0
            for t in X_UNITS:
                xt = xpool.tile(
                    [P, t, CHUNK], f16, tag=f"x{t}",
                    bufs=(3 if t == 8 else 2),
                )
                src = x_d[row * P : (row + t) * P, :]
                nc.sync.dma_start(
                    xt[:], src.rearrange("(p t) c -> p t c", p=P)
                )
                n_act = ACT_TILES[t]
                if n_act:
                    nc.vector.tensor_mul(
                        xt[:, 0:n_act, :],
                        xt[:, 0:n_act, :],
                        wsum16[:].unsqueeze(1).broadcast_to((P, n_act, CHUNK)),
                    )
                for k in range(n_act):
                    nc.scalar.activation(
                        act_out[:],
                        xt[:, k, :],
                        mybir.ActivationFunctionType.Copy,
                        bias=0.0,
                        scale=1.0,
                        accum_out=s_col(row + k),
                    )
                for k in range(n_act, t):
                    nc.vector.scalar_tensor_tensor(
                        scratch[:],
                        xt[:, k, :],
                        0.0,
                        wsum16[:],
                        op0=mybir.AluOpType.bypass,
                        op1=mybir.AluOpType.mult,
                        accum_out=s_col(row + k),
                    )
                prev_row, row = row, row + t
                if prev_row < 16 <= row:
                    # Row-tiles 0-15 complete: store them while the tail
                    # still streams (Activation HWDGE queue keeps SP free).
                    nc.scalar.dma_start(out_d[:, 0:16], s_a[:])
            assert row == X_TILES

            nc.scalar.dma_start(out_d[:, 16:X_TILES], s_b[:])

    nc.compile()
    return nc


def _get_nc():
    global _compiled_nc
    if _compiled_nc is None:
        _compiled_nc = _build_nc()
    return _compiled_nc


def kernel(x: np.ndarray, weight: np.ndarray) -> np.ndarray:
    from concourse.bass_utils import run_bass_kernel_spmd

    x = np.asarray(x)
    weight = np.asarray(weight)
    assert x.shape == (B, IN) and weight.shape == (HID, IN)
    x16 = x.astype(np.float16)
    w16 = weight.astype(np.float16)

    nc = _get_nc()
    in_maps = [
        {
            "x": np.ascontiguousarray(x16[:, d * CHUNK : (d + 1) * CHUNK]),
            "w": np.ascontiguousarray(w16[:, d * CHUNK : (d + 1) * CHUNK]),
        }
        for d in range(N_CORES)
    ]
    res = run_bass_kernel_spmd(nc, in_maps, core_ids=list(range(N_CORES)))

    # Column (r_u + k) of the [P, 32] output holds, at partition p, the
    # sum for batch row R_u + p*t_u + k (the "(p t)" DMA layout).
    b_of_col = np.empty((X_TILES, P), dtype=np.int64)
    rcol, R = 0, 0
    for t in X_UNITS:
        for k in range(t):
            b_of_col[rcol + k] = R + np.arange(P) * t + k
        rcol += t
        R += t * P
    acc = np.zeros(B, dtype=np.float64)
    for d in range(N_CORES):
        o = res.results[d]["out"].astype(np.float64)  # [P, X_TILES]
        for c in range(X_TILES):
            acc[b_of_col[c]] += o[:, c]
    return acc.reshape(B, 1).astype(np.float32)


# revision 18
# speedup vs baseline: 1.1485x; 1.0302x over previous
"""Trainium2 Bass kernel for nn_ModelNew_78847009620052 (dense_mlp).

Computes, for x [4096, 8192] and weight [8192, 8192]:
    out[b, 0] = 0.75 * sum_i x[b, i] * (sum_j weight[j, i])
(which equals 1.5 * sum(x @ W.T / 2, axis=1, keepdims=True)).

Sharding: column-shard the contraction dim IN=8192 into 8 chunks of 1024.
Core d receives x[:, d*1024:(d+1)*1024] and weight[:, d*1024:(d+1)*1024],
produces a partial [128, 32] result; host sums the 8 partials (after a
[128,32] -> [4096,1] reindex).

The kernel is a pure HBM-bandwidth problem (every input byte is read
once, ~408 GB/s/core sustained).  The single biggest lever is the byte
count: the host casts both inputs to float16 before upload, halving
per-core traffic from 48MB to 24MB (~123us -> ~62us of streaming).
Accumulation stays in fp32 (PSUM matmul accumulate + fp32 accum_out),
so the end-to-end error is ~5e-4 relative -- 40x inside the 2e-2 gate
(fp16 keeps 11 mantissa bits; x, w ~ N(0,1), all intermediates are
orders of magnitude below fp16's 65504 max).

Per-core device algorithm:
  Phase 1 (weight, 16MB fp16): stream [P, t, 1024] tiles (t<=8, 2MB
    DMAs).  Per pair of row-tiles: ONE VectorE fp16 in-tile add folds
    the pair, then ONE TensorE fp16 matmul pair (ones*0.75 stationary)
    accumulates the folded tile's column sums into PSUM fp32, broadcast
    to all 128 partitions.  No cross-unit add trees: every unit's
    serial chain is land -> add -> matmul pair, so the reduction never
    lags the DMA stream.  The last two units are single row-tiles
    consumed by a direct matmul pair, keeping the stop chain minimal.
    After the stop matmul one VectorE copy casts the PSUM column sums
    to an SBUF fp16 operand tile.
  Phase 2 (x, 8MB fp16): every ACCUMULATING op on this HW runs at
    ~1 elem/cycle/lane regardless of dtype (DVE scalar_tensor_tensor
    ~1.22us, DVE reduce_sum ~1.05ns/elem, ScalarE activation-accum
    ~1.33us per [128,1024] tile), while plain fp16 elementwise ops run
    at 2x.  GpSimd is useless here: its tensor ops are 2.5x slower AND
    fight VectorE for the shared SBUF port (measured: concurrent gp
    muls degrade DVE STTs 1.22us -> 3.0us).  So the row sums are split
    across the two usable engines:
      - leading tiles per unit: one fused in-place DVE fp16 mul
        (x *= colsums, ~0.53ns/elem) + one ScalarE activation-accum
        per tile;
      - trailing tiles: one DVE scalar_tensor_tensor (fused
        mul+reduce) per tile, straight from the raw x tile;
    with the tile split chosen so both engines finish together
    (ScalarE starts ~8us later, gated by the first mul).

All DMAs are HWDGE, weight queued strictly before x, so the SDMA
engines drain the weight stream at full HBM rate and the x stream
follows seamlessly.  Output stores are issued from the Activation
HWDGE queue so the in-order SP sequencer never blocks later x DMA
issues on a store's semaphore wait.  Small tail units keep the
post-stream serial chain (last STT -> store -> teardown) short.
"""

import numpy as np

B, IN, HID = 4096, 8192, 8192
N_CORES = 8
CHUNK = IN // N_CORES          # 1024 columns per core
SCALE = 1.5 / 2.0              # 0.75, folded into the ones stationary
P = 128                        # partitions
W_TILES = HID // P             # 64 weight row-tiles per core
X_TILES = B // P               # 32 x row-tiles per core

# Row-tiles per DMA ([P, t, 1024] fp16 = t * 256KB per DMA).  Tiles are
# loaded "(p t) c -> p t c": partition p holds t CONTIGUOUS rows
# (p*t+k), so every DMA descriptor line is a single t*2KB contiguous
# run (vs 2KB strided runs with the (t p) layout) — measurably better
# HBM efficiency.  The host gather undoes the batch permutation.
# The x stream leads with a small unit so the first VectorE STTs start
# as soon as the column sums are ready instead of waiting ~5us for a
# full 2MB tile.
W_UNITS = [4, 4] + [8] * 6 + [4, 2, 1, 1]
X_UNITS = [2, 8, 8, 8, 4, 1, 1]
assert sum(W_UNITS) == W_TILES and sum(X_UNITS) == X_TILES

S_SPLIT = 24                   # first store covers s columns [0, 24)

_compiled_nc = None


def _build_nc():
    import concourse.bass as bass
    import concourse.tile as tile
    from concourse import bacc, mybir

    f32 = mybir.dt.float32
    f16 = mybir.dt.float16
    nc = bacc.Bacc(
        "TRN2",
        target_bir_lowering=False,
        debug=False,
        num_devices=N_CORES,
    )

    x_d = nc.dram_tensor("x", [B, CHUNK], f16, kind="ExternalInput")
    w_d = nc.dram_tensor("w", [HID, CHUNK], f16, kind="ExternalInput")
    out_d = nc.dram_tensor("out", [P, X_TILES], f32, kind="ExternalOutput")

    with tile.TileContext(nc) as tc:
        with (
            tc.tile_pool(name="wpool", bufs=3) as wpool,
            tc.tile_pool(name="xpool", bufs=3) as xpool,
            tc.tile_pool(name="const", bufs=1) as const,
            tc.tile_pool(name="psum", bufs=1, space="PSUM") as psum_pool,
        ):
            # Warmup: tiny DMA issued first to probe/shrink HWDGE cold start.
            warm = const.tile([P, 16], f16)
            nc.sync.dma_start(warm[:], w_d[0:P, 0:16])

            ones = const.tile([P, P], f16)
            nc.vector.memset(ones[:], SCALE)

            # Column sums accumulate in PSUM fp32, broadcast to all 128
            # partitions by the ones matmul.
            psum_bc = psum_pool.tile([P, CHUNK], f32, tag="psum_bc")

            first_mm = [True]

            def colsum_pair(src_ap, stop):
                for h in range(2):
                    nc.tensor.matmul(
                        psum_bc[:, h * 512 : (h + 1) * 512],
                        ones[:],
                        src_ap[:, h * 512 : (h + 1) * 512],
                        start=first_mm[0],
                        stop=stop,
                    )
                first_mm[0] = False

            # --- Phase 1: weight stream. Per pair of row-tiles: one DVE
            # fold + one PE pair; single-row-tile units go straight to PE.
            row = 0
            for ui, t in enumerate(W_UNITS):
                last_unit = ui == len(W_UNITS) - 1
                wt = wpool.tile(
                    [P, t, CHUNK], f16, tag=f"w{t}",
                    bufs=(3 if t == 8 else 2),
                )
                src = w_d[row * P : (row + t) * P, :]
                nc.sync.dma_start(
                    wt[:], src.rearrange("(p t) c -> p t c", p=P)
                )
                if t == 1:
                    colsum_pair(wt[:, 0, :], stop=last_unit)
                else:
                    # Fold pairs, then pairs-of-pairs, on DVE (fp16 2x
                    # rate); PE consumes one colsum pair per 4 row-tiles.
                    # The faster contiguous-layout stream made PE's
                    # matmul backlog the wsum critical path; folding
                    # deeper halves PE work for +9us of idle DVE time.
                    for k in range(t // 2):
                        nc.vector.tensor_add(
                            wt[:, 2 * k, :], wt[:, 2 * k, :], wt[:, 2 * k + 1, :]
                        )
                    for k in range(t // 4):
                        nc.vector.tensor_add(
                            wt[:, 4 * k, :], wt[:, 4 * k, :], wt[:, 4 * k + 2, :]
                        )
                        colsum_pair(wt[:, 4 * k, :], stop=False)
                    if t == 2:
                        colsum_pair(wt[:, 0, :], stop=False)
                row += t
            assert row == W_TILES

            # Cast the broadcast column sums to an SBUF fp16 operand so
            # phase-2 STTs run at the 16-bit DVE rate.
            wsum16 = const.tile([P, CHUNK], f16)
            nc.vector.tensor_copy(wsum16[:], psum_bc[:])

            # --- Phase 2: x stream. Every accumulating op on this HW runs
            # at ~1 elem/cycle/lane regardless of dtype (measured: DVE STT
            # ~1.22us, DVE reduce ~1.05ns/elem, ACT accum ~1.33us per
            # [128,1024] tile), while plain fp16 elementwise ops run at 2x
            # (~0.6ns/elem). The optimal 2-engine split is therefore:
            #   per unit, the leading n_act tiles: ONE fused in-place DVE
            #     mul (x *= colsums, ~0.53ns/elem), then one ScalarE
            #     activation-accum per tile (1.33us);
            #   the trailing tiles: one DVE STT (fused mul+reduce, 1.22us)
            #     each, straight from the raw x tile.
            # 14 ACT tiles / 18 STT tiles makes both engines finish
            # together (ACT starts later; see ACT_TILES below).
            s_a = const.tile([P, 16], f32)            # row-tiles 0-15
            s_b = const.tile([P, X_TILES - 16], f32)  # row-tiles 16-31
            act_out = const.tile([P, CHUNK], f16)
            scratch = const.tile([P, CHUNK], f16)

            def s_col(idx):
                return (
                    s_a[:, idx : idx + 1]
                    if idx < 16
                    else s_b[:, idx - 16 : idx - 15]
                )

            # Per-unit count of leading tiles reduced on ScalarE
            # (remainder go through DVE STT). ACT starts ~8us after DVE
            # (it waits on the first mul), so it carries fewer tiles for
            # both engines to finish together.
            ACT_TILES = {8: 4, 4: 2, 2: 2, 1: 0}

            row = 0
            for t in X_UNITS:
                xt = xpool.tile(
                    [P, t, CHUNK], f16, tag=f"x{t}",
                    bufs=(3 if t == 8 else 2),
                )
                src = x_d[row * P : (row + t) * P, :]
                nc.sync.dma_start(
                    xt[:], src.rearrange("(p t) c -> p t c", p=P)
                )
                n_act = ACT_TILES[t]
                if n_act:
                    nc.vector.tensor_mul(
                        xt[:, 0:n_act, :],
                        xt[:, 0:n_act, :],
                        wsum16[:].unsqueeze(1).broadcast_to((P, n_act, CHUNK)),
                    )
                for k in range(n_act):
                    nc.scalar.activation(
                        act_out[:],
                        xt[:, k, :],
                        mybir.ActivationFunctionType.Copy,
                        bias=0.0,
                        scale=1.0,
                        accum_out=s_col(row + k),
                    )
                for k in range(n_act, t):
                    nc.vector.scalar_tensor_tensor(
                        scratch[:],
                        xt[:, k, :],
                        0.0,
                        wsum16[:],
                        op0=mybir.AluOpType.bypass,
                        op1=mybir.AluOpType.mult,
                        accum_out=s_col(row + k),
                    )
                prev_row, row = row, row + t
                if prev_row < 16 <= row:
                    # Row-tiles 0-15 complete: store them while the tail
                    # still streams (Activation HWDGE queue keeps SP free).
                    nc.scalar.dma_start(out_d[:, 0:16], s_a[:])
            assert row == X_TILES

            nc.scalar.dma_start(out_d[:, 16:X_TILES], s_b[:])

    nc.compile()
    return nc


def _get_nc():
    global _compiled_nc
    if _compiled_nc is None:
        _compiled_nc = _build_nc()
    return _compiled_nc


def kernel(x: np.ndarray, weight: np.ndarray) -> np.ndarray:
    from concourse.bass_utils import run_bass_kernel_spmd

    x = np.asarray(x)
    weight = np.asarray(weight)
    assert x.shape == (B, IN) and weight.shape == (HID, IN)
    x16 = x.astype(np.float16)
    w16 = weight.astype(np.float16)

    nc = _get_nc()
    in_maps = [
        {
            "x": np.ascontiguousarray(x16[:, d * CHUNK : (d + 1) * CHUNK]),
            "w": np.ascontiguousarray(w16[:, d * CHUNK : (d + 1) * CHUNK]),
        }
        for d in range(N_CORES)
    ]
    res = run_bass_kernel_spmd(nc, in_maps, core_ids=list(range(N_CORES)))

    # Column (r_u + k) of the [P, 32] output holds, at partition p, the
    # sum for batch row R_u + p*t_u + k (the "(p t)" DMA layout).
    b_of_col = np.empty((X_TILES, P), dtype=np.int64)
    rcol, R = 0, 0
    for t in X_UNITS:
        for k in range(t):
            b_of_col[rcol + k] = R + np.arange(P) * t + k
        rcol += t
        R += t * P
    acc = np.zeros(B, dtype=np.float64)
    for d in range(N_CORES):
        o = res.results[d]["out"].astype(np.float64)  # [P, X_TILES]
        for c in range(X_TILES):
            acc[b_of_col[c]] += o[:, c]
    return acc.reshape(B, 1).astype(np.float32)
